# revision 39
# baseline (speedup 1.0000x reference)
"""Supervised-contrastive point-cloud loss on Trainium2 (8 NeuronCores).

Inputs (full): features [8, 128, 4096] f32, labels_all [8, 4096] int32.
Sharding: data-parallel over the batch dim - core b handles cloud b.

Host prep (per cloud): sort points by label (loss is a mean over points,
so permutation-invariant), L2-normalize columns, cast to bf16.  With
sorted labels every class occupies a contiguous segment of length
<= 385 (asserted), so each point's same-class partners all lie within
3 blocks (384 rows) of its own 512-wide column chunk.

Device (per core), exploiting dp symmetry (dp = exp(10 * vhat^T vhat)):
Chunks are processed in double-chunk windows (a=2w, b=2w+1) so each
128-row stationary is loaded once and streams two back-to-back 512-col
matmuls (the PE only sustains full rate without weight swaps).
For block row m (0 .. min(4b+6, 31)):
  PE:  G(m,a), G(m,b) into a [128,2,512] PSUM tile     (skip a if m>4a+6)
  DVE: diag blocks: G -= 1e5*I  => exp underflows to 0 on the diagonal
  ACT: dp = exp(10 G) -> bf16 SBUF window [C, 32, 2, 512]
  PE:  CS[c, x] += y17_m^T dp(m,.)  into per-chunk [17,512] PSUM
       (y17 = per-block one-hot labels + a ones row -> row 16 = colsum)
  far-below-diagonal blocks skip CS; GpSimd accumulates their tiles
  elementwise (bf16) and one ones-matmul folds the result into cs[16].
CS[c, x] = sum over rows p < 512h+896 with label c of dp[p, x], so
  positives_x = CS[label_x, x]
  totals_x    = CS[16, x]  +  sum over cols >= 512h+896 of row x
The second term is block-aligned row-direction reduces of already-
computed upper tiles (symmetry: row x of dp = column x) on DVE; two
same-window tiles reduce in one strided tensor_reduce (axis=XY).
Host tail: gather, log, mean  (O(N) numpy).
"""

import contextlib
import sys

for _p in ("/opt/trn_rl_repo",):
    if _p not in sys.path:
        sys.path.append(_p)

import numpy as np
import ml_dtypes

import concourse.bass as bass  # noqa: F401
import concourse.bacc as bacc
import concourse.tile as tile
from concourse import mybir
from concourse.bass_utils import run_bass_kernel_spmd

F32 = mybir.dt.float32
BF16 = mybir.dt.bfloat16
AF = mybir.ActivationFunctionType
ALU = mybir.AluOpType
AX = mybir.AxisListType

B, C, N = 8, 128, 4096
NCLS = 16
NROW = NCLS + 1          # 16 class rows + colsum row
NBLK = N // 128          # 32 block rows
NCH = N // 512           # 8 column chunks
NW = NCH // 2            # 4 double-chunk windows
TEMP_INV = 10.0
BIGDIAG = 1.0e5
MAXSEG = 385             # max class segment length the band covers

# tiles per chunk: block rows 0 .. min(4h+6, 31)
TPC = [min(4 * h + 7, NBLK) for h in range(NCH)]
# blocks whose column sums go through GpSimd accumulation instead of CS
# (must be far below the diagonal in BOTH chunks of the window: m <= 4a-4)
NB_GP = [0, 5, 9, 10]    # count per window; blocks 0..n-1


def _slot_plan():
    """Row-direction reduce pieces per block row m (hd = m//4):
    partial [384:512] of tile (m, hd+1), then full tiles (m, hd+2..7).
    Two fulls in the same window reduce in ONE tensor_reduce (axis=XY).
    piece = (kind, m, h, h2, slot); kind: 'p' partial, 't' pair, 's' single.
    """
    pieces = []
    slot = 0
    for m in range(NBLK):
        hd = m // 4
        if hd + 1 < NCH:
            pieces.append(("p", m, hd + 1, None, slot)); slot += 1
        fulls = list(range(hd + 2, NCH))
        for w in range(NW):
            hs = [h for h in fulls if h // 2 == w]
            if len(hs) == 2:
                pieces.append(("t", m, hs[0], hs[1], slot)); slot += 1
            elif len(hs) == 1:
                pieces.append(("s", m, hs[0], None, slot)); slot += 1
    return slot, pieces


NSLOT, PIECES = _slot_plan()
RSW = ((NSLOT + 3) // 4) * 4  # pad rsout width


def build_program():
    nc = bacc.Bacc("TRN2", target_bir_lowering=False, debug=False, num_devices=B)

    vhat_d = nc.dram_tensor("vhat", [C, N], BF16, kind="ExternalInput").ap()
    y17_d = nc.dram_tensor("y17", [C, NBLK * NROW], BF16, kind="ExternalInput").ap()
    negeye_d = nc.dram_tensor("negeye", [128, 128], BF16, kind="ExternalInput").ap()
    sheye_d = nc.dram_tensor("sheye", [128, 896], BF16, kind="ExternalInput").ap()
    cs_d = nc.dram_tensor("csout", [NROW, N], F32, kind="ExternalOutput").ap()
    rs_d = nc.dram_tensor("rsout", [128, RSW], F32, kind="ExternalOutput").ap()

    due = {}
    for kind, m, h1, h2, slot in PIECES:
        h_last = h1 if h2 is None else h2
        due.setdefault((h_last // 2, m), []).append((kind, m, h1, h2, slot))

    with tile.TileContext(nc) as tc, contextlib.ExitStack() as _stack:
        with (
            tc.tile_pool(name="const", bufs=1) as constp,
            tc.tile_pool(name="dp", bufs=2) as dpp,
            tc.tile_pool(name="cssb", bufs=2) as cssbp,
            tc.tile_pool(name="pg", bufs=3, space="PSUM") as pgp,
            tc.tile_pool(name="pcs", bufs=1, space="PSUM") as pcsp,
        ):
            # ---- constants in (two DMA queues: sync + gpsimd) ----
            y17_sb = constp.tile([C, NBLK * NROW], BF16)
            nc.sync.dma_start(y17_sb[:], y17_d[:])
            negeye_sb = constp.tile([128, 128], BF16)
            nc.gpsimd.dma_start(negeye_sb[:], negeye_d[:])
            sheye_sb = constp.tile([128, 896], BF16)
            nc.gpsimd.dma_start(sheye_sb[:], sheye_d[:])
            vhat_sb = constp.tile([C, N], BF16)
            for p in range(4):
                sl = slice(p * 1024, (p + 1) * 1024)
                eng = nc.sync if p % 2 == 0 else nc.gpsimd
                eng.dma_start(vhat_sb[:, sl], vhat_d[:, sl])

            rs_sb = constp.tile([128, RSW], F32)
            nc.vector.memset(rs_sb[:], 0.0)

            # warm the Exp activation table during the DMAs
            warm = constp.tile([1, 1], F32)
            nc.vector.memset(warm[:], 0.0)
            warm2 = constp.tile([1, 1], F32)
            nc.scalar.activation(warm2[:], warm[:], AF.Exp)

            # GpSimd column-sum accumulators (bf16), one per chunk of w>=1
            accs = {}
            for h in range(2, NCH):
                acc_t = constp.tile([128, 512], BF16, name=f"acc{h}")
                nc.gpsimd.memset(acc_t[:], 0.0)
                accs[h] = acc_t

            pending = []          # (dp_window_tile, piece) reduce queue

            def drain_reduces(k):
                for _ in range(min(k, len(pending))):
                    dpw, (kind, m, h1, h2, slot) = pending.pop(0)
                    acc = rs_sb[:, slot:slot + 1]
                    if kind == "p":
                        nc.vector.tensor_reduce(
                            acc, dpw[:, m, h1 % 2, 384:512],
                            axis=AX.X, op=ALU.add,
                        )
                    elif kind == "s":
                        nc.vector.tensor_reduce(
                            acc, dpw[:, m, h1 % 2, :],
                            axis=AX.X, op=ALU.add,
                        )
                    else:
                        nc.vector.tensor_reduce(
                            acc, dpw[:, m, :, :],
                            axis=AX.XY, op=ALU.add,
                        )

            carry = []            # CS/evac work carried into the next window

            for w in range(NW):
                ha, hb = 2 * w, 2 * w + 1
                Ta, Tb = TPC[ha], TPC[hb]
                nb = NB_GP[w]     # blocks 0..nb-1 bypass CS via GpSimd
                csa = slice(ha * 512, (ha + 1) * 512)
                csb = slice(hb * 512, (hb + 1) * 512)
                cs_a = pcsp.tile([NROW, 512], F32, tag="csA")
                cs_b = pcsp.tile([NROW, 512], F32, tag="csB")
                dp_sb = dpp.tile([C, NBLK, 2, 512], BF16, tag="dp")

                def emit_cs(m, cs_a=cs_a, cs_b=cs_b, dp_sb=dp_sb,
                            Ta=Ta, Tb=Tb, nb=nb):
                    if m < nb:
                        return
                    lhs = y17_sb[:, m * NROW:(m + 1) * NROW]
                    if m < Ta:
                        nc.tensor.matmul(
                            cs_a[:], lhs, dp_sb[:, m, 0, :],
                            start=(m == nb),
                            stop=(m == Ta - 1),
                        )
                    nc.tensor.matmul(
                        cs_b[:], lhs, dp_sb[:, m, 1, :],
                        start=(m == nb),
                        stop=(m == Tb - 1),
                    )

                def emit_evac(cs_a=cs_a, cs_b=cs_b, csa=csa, csb=csb):
                    cs_sba = cssbp.tile([NROW, 512], F32, tag="cssb")
                    nc.vector.tensor_copy(cs_sba[:], cs_a[:])
                    nc.sync.dma_start(cs_d[:, csa], cs_sba[:])
                    cs_sbb = cssbp.tile([NROW, 512], F32, tag="cssb")
                    nc.vector.tensor_copy(cs_sbb[:], cs_b[:])
                    nc.sync.dma_start(cs_d[:, csb], cs_sbb[:])

                for m in range(Tb):
                    # diagonal square membership (at most one parity)
                    dpar = doff = None
                    if 4 * ha <= m <= 4 * ha + 3:
                        dpar, doff = 0, (m - 4 * ha)
                    elif 4 * hb <= m <= 4 * hb + 3:
                        dpar, doff = 1, (m - 4 * hb)
                    gpm = pgp.tile([128, 2, 512], F32, tag="g")
                    lhs = vhat_sb[:, m * 128:(m + 1) * 128]
                    if m < Ta:
                        nc.tensor.matmul(gpm[:, 0, :], lhs, vhat_sb[:, csa],
                                         start=True, stop=(dpar != 0))
                    nc.tensor.matmul(gpm[:, 1, :], lhs, vhat_sb[:, csb],
                                     start=True, stop=(dpar != 1))
                    if dpar is not None:
                        # kill the diagonal: accumulate -1e5 * shifted eye
                        nc.tensor.matmul(
                            gpm[:, dpar, :], negeye_sb[:],
                            sheye_sb[:, 384 - doff * 128:896 - doff * 128],
                            start=False, stop=True,
                        )
                    if m < len(carry):
                        carry[m]()           # previous window's CS tail/evac
                    if m >= 2:
                        emit_cs(m - 2)
                    if m < Ta:
                        nc.scalar.activation(
                            dp_sb[:, m, :, :], gpm[:, :, :],
                            AF.Exp, scale=TEMP_INV,
                        )
                    else:
                        nc.scalar.activation(
                            dp_sb[:, m, 1, :], gpm[:, 1, :],
                            AF.Exp, scale=TEMP_INV,
                        )
                    if m < nb:
                        for par, hh in ((0, ha), (1, hb)):
                            if hh in accs:
                                nc.gpsimd.tensor_tensor(
                                    accs[hh][:], accs[hh][:],
                                    dp_sb[:, m, par, :], op=ALU.add)
                    pending.extend((dp_sb, p) for p in due.get((w, m), []))
                    drain_reduces(2)
                def emit_fold(cs_a=cs_a, cs_b=cs_b, nb=nb, ha=ha, hb=hb):
                    if nb == 0:
                        return
                    ones = y17_sb[:, 0:1]   # block 0 ones column (row 0)
                    nc.tensor.matmul(cs_a[0:1, :], ones, accs[ha][:],
                                     start=False, stop=True,
                                     skip_group_check=True)
                    nc.tensor.matmul(cs_b[0:1, :], ones, accs[hb][:],
                                     start=False, stop=True,
                                     skip_group_check=True)

                carry = [
                    lambda f=emit_cs, mm=Tb - 2: f(mm),
                    lambda f=emit_cs, mm=Tb - 1: f(mm),
                    emit_fold,
                    emit_evac,
                ]

            for f in carry:
                f()
            drain_reduces(len(pending))
            nc.sync.dma_start(rs_d[:], rs_sb[:])

    nc.compile()
    return nc


_NC = None


def _get_program():
    global _NC
    if _NC is None:
        _NC = build_program()
    return _NC


def make_in_maps(features, labels_all):
    feats = np.asarray(features, dtype=np.float32)
    labels = np.asarray(labels_all, dtype=np.int64)
    negeye = (np.eye(128) * -BIGDIAG).astype(ml_dtypes.bfloat16)
    sheye = np.zeros((128, 896), dtype=ml_dtypes.bfloat16)
    sheye[np.arange(128), np.arange(128) + 384] = 1.0
    in_maps = []
    orders = []
    for b in range(B):
        order = np.argsort(labels[b], kind="stable")
        orders.append(order)
        lab = labels[b][order]
        cnt = np.bincount(lab, minlength=NCLS)
        assert cnt.max() <= MAXSEG, f"class segment {cnt.max()} > {MAXSEG}"
        f = feats[b][:, order]
        nrm = np.sqrt((f.astype(np.float64) ** 2).sum(axis=0))
        nrm = np.maximum(nrm, 1e-12)
        vhat = (f / nrm).astype(ml_dtypes.bfloat16)
        y17 = np.zeros((C, NBLK * NROW), dtype=ml_dtypes.bfloat16)
        blk = np.arange(N) // 128
        row = np.arange(N) % 128
        # row 0 of each block = ones (colsum); classes at rows 1..16
        y17[row, blk * NROW + 1 + lab] = 1.0
        y17[:, np.arange(NBLK) * NROW] = 1.0
        in_maps.append({"vhat": vhat, "y17": y17,
                        "negeye": negeye, "sheye": sheye})
    return in_maps, orders, labels


def finish_on_host(results, orders, labels):
    slots_of_m = [[] for _ in range(NBLK)]
    for kind, m, h1, h2, slot in PIECES:
        slots_of_m[m].append(slot)
    losses = []
    for b in range(B):
        cs = np.asarray(results[b]["csout"], dtype=np.float64)   # [17, N]
        rs = np.asarray(results[b]["rsout"], dtype=np.float64)   # [128, RSW]
        lab = labels[b][orders[b]]
        pos = cs[1 + lab, np.arange(N)]
        tot = cs[0]
        m = np.arange(N) // 128
        row = np.arange(N) % 128
        extra = np.zeros(N)
        for mm in range(NBLK):
            sel = m == mm
            if slots_of_m[mm]:
                extra[sel] = rs[row[sel]][:, slots_of_m[mm]].sum(axis=1)
        tot = tot + extra
        dev = np.log(tot) - np.log(pos)
        losses.append(dev.mean())
    return np.asarray(np.float32(np.mean(losses)))


def run(features, labels_all, **spmd_kwargs):
    nc = _get_program()
    in_maps, orders, labels = make_in_maps(features, labels_all)
    res = run_bass_kernel_spmd(nc, in_maps, list(range(B)), **spmd_kwargs)
    out = finish_on_host(res.results, orders, labels)
    return out, res


def kernel(features, labels_all):
    out, _ = run(features, labels_all)
    return out
